# revision 1
# baseline (speedup 1.0000x reference)
"""MoE layer (B=8, T=2048, D=512, F=2048, E=16, top-2) on 8 TRN2 NeuronCores.

kernel(**inputs) takes the full unsharded inputs (as in setup_inputs()) and
returns (output (B,T,D) f32, aux_loss scalar f32) matching the reference.

Strategy (expert-parallel with host-side token dispatch, 2 HW launches):
  Launch 1 (router, token-parallel): each core computes fp32 logits for its
    2048-token shard: logitsT (E, 2048) = Wr^T @ x_shard^T on the PE array.
  Host: softmax / top-2 / gates / aux_loss in fp32; build per-expert token
    lists; gather tokens into per-expert capacity-padded transposed bf16
    buffers ("all-to-all by top-k indices" done as a host gather).
  Launch 2 (experts, expert-parallel): core m handles two experts (one large,
    one small, paired for load balance); computes
    yT = W2^T @ gelu(W1^T @ xgT + b1) + b2 with bf16 matmuls accumulating in
    fp32 PSUM; everything stays transposed (tokens on the matmul free dim) so
    no on-device transposes are needed.
  Host: scatter-add gated expert outputs back to token order.

Tokens routed beyond a slot's capacity (never expected for this input
distribution) are computed exactly on the host as a fallback.
"""

import os
import sys

import numpy as np
import ml_dtypes

for _p in ("/opt/trn_rl_repo",):
    if os.path.isdir(_p) and _p not in sys.path:
        sys.path.append(_p)

import concourse.bacc as bacc
import concourse.bass as bass
import concourse.mybir as mybir
import concourse.tile as tile
from concourse.bass_utils import run_bass_kernel_spmd

# ---------------------------------------------------------------- constants
B, T, D, F, E, K = 8, 2048, 512, 2048, 16, 2
N_CORES = 8
NT = B * T                    # 16384 tokens
TOK_PER_CORE = NT // N_CORES  # 2048
EXP_PER_CORE = E // N_CORES   # 2
# Asymmetric per-slot capacities: host assigns the 8 largest experts to
# slot 0 and the 8 smallest to slot 1, so every core gets one big + one
# small expert. Expected counts are ~2048 +- 350 for this router init.
CAP0 = 2560
CAP1 = 2048
CAPS = (CAP0, CAP1)
CHUNK = 512                   # token chunk (matmul free dim)
P = 128
DP = D // P                   # 4  D-chunks of 128
FP = F // P                   # 16 F-chunks of 128

FP32 = mybir.dt.float32
BF16 = mybir.dt.bfloat16


# ---------------------------------------------------------------- router
def build_router():
    """Per core: logitsT (E, TOK_PER_CORE) = Wr^T @ xT_shard, all fp32."""
    nc = bacc.Bacc()
    xT = nc.declare_dram_parameter("xT", [DP, P, TOK_PER_CORE], FP32, isOutput=False)
    wr = nc.declare_dram_parameter("wr", [DP, P, E], FP32, isOutput=False)
    logitsT = nc.declare_dram_parameter("logitsT", [E, TOK_PER_CORE], FP32, isOutput=True)

    with tile.TileContext(nc) as tc:
        with (
            tc.tile_pool(name="wpool", bufs=1) as wpool,
            tc.tile_pool(name="xpool", bufs=3) as xpool,
            tc.tile_pool(name="opool", bufs=3) as opool,
            tc.tile_pool(name="psum", bufs=2, space="PSUM") as psum_pool,
        ):
            wr_t = wpool.tile([P, DP * E], FP32)
            for d in range(DP):
                nc.sync.dma_start(wr_t[:, d * E:(d + 1) * E], wr[d])
            for c in range(TOK_PER_CORE // CHUNK):
                xt = [xpool.tile([P, CHUNK], FP32, tag=f"x{d}", name=f"xt{d}")
                      for d in range(DP)]
                for d in range(DP):
                    nc.sync.dma_start(xt[d][:], xT[d, :, bass.ts(c, CHUNK)])
                ps = psum_pool.tile([E, CHUNK], FP32)
                for d in range(DP):
                    nc.tensor.matmul(
                        ps[:], wr_t[:, d * E:(d + 1) * E], xt[d][:],
                        start=(d == 0), stop=(d == DP - 1),
                    )
                ot = opool.tile([E, CHUNK], FP32)
                nc.scalar.copy(ot[:], ps[:])
                nc.sync.dma_start(logitsT[:, bass.ts(c, CHUNK)], ot[:])
    return nc


# ---------------------------------------------------------------- experts
def build_experts():
    """Per core: for each expert slot, yT = W2^T @ gelu(W1^T @ xgT + b1) + b2.

    Inputs (per core):
      xg{s} (DP, P, CAPS[s])  bf16  gathered tokens, transposed (D-major)
      w1    (2, DP, P, F)     bf16  W1 natural layout (D, F), D on partitions
      w2    (2, FP, P, D)     bf16  W2 natural layout (F, D), F on partitions
      b1r   (2, P, FP)        f32   b1r[e, p, f] = b1[e, f*128+p]
      b2r   (2, P, DP)        f32   likewise
    Outputs:
      yT{s} (DP, P, CAPS[s])  bf16
    """
    nc = bacc.Bacc()
    xgs, yTs = [], []
    for sl, cap in enumerate(CAPS):
        xgs.append(nc.declare_dram_parameter(f"xg{sl}", [DP, P, cap], BF16, isOutput=False))
        yTs.append(nc.declare_dram_parameter(f"yT{sl}", [DP, P, cap], BF16, isOutput=True))
    w1 = nc.declare_dram_parameter("w1", [EXP_PER_CORE, DP, P, F], BF16, isOutput=False)
    w2 = nc.declare_dram_parameter("w2", [EXP_PER_CORE, FP, P, D], BF16, isOutput=False)
    b1r = nc.declare_dram_parameter("b1r", [EXP_PER_CORE, P, FP], FP32, isOutput=False)
    b2r = nc.declare_dram_parameter("b2r", [EXP_PER_CORE, P, DP], FP32, isOutput=False)

    gelu = mybir.ActivationFunctionType.Gelu_apprx_tanh

    with tile.TileContext(nc) as tc:
        with (
            tc.tile_pool(name="w1pool", bufs=2) as w1pool,
            tc.tile_pool(name="w2pool", bufs=2) as w2pool,
            tc.tile_pool(name="bpool", bufs=2) as bpool,
            tc.tile_pool(name="xpool", bufs=4) as xpool,
            tc.tile_pool(name="hpool", bufs=3) as hpool,
            tc.tile_pool(name="ypool", bufs=3) as ypool,
            tc.tile_pool(name="psum_h", bufs=6, space="PSUM") as psum_h,
            tc.tile_pool(name="psum_y", bufs=2, space="PSUM") as psum_y,
        ):
            for e in range(EXP_PER_CORE):
                xg, yT, cap = xgs[e], yTs[e], CAPS[e]
                w1t = [w1pool.tile([P, F], BF16, tag=f"w1_{d}", name=f"w1t{d}")
                       for d in range(DP)]
                for d in range(DP):
                    nc.sync.dma_start(w1t[d][:], w1[e, d])
                w2t = [w2pool.tile([P, D], BF16, tag=f"w2_{f}", name=f"w2t{f}")
                       for f in range(FP)]
                for f in range(FP):
                    nc.sync.dma_start(w2t[f][:], w2[e, f])
                b1t = bpool.tile([P, FP], FP32, tag="b1")
                nc.sync.dma_start(b1t[:], b1r[e])
                b2t = bpool.tile([P, DP], FP32, tag="b2")
                nc.sync.dma_start(b2t[:], b2r[e])

                for c in range(cap // CHUNK):
                    xt = [xpool.tile([P, CHUNK], BF16, tag=f"x{d}", name=f"xt{d}")
                          for d in range(DP)]
                    for d in range(DP):
                        nc.sync.dma_start(xt[d][:], xg[d, :, bass.ts(c, CHUNK)])
                    # H^T tiles: h[f] (128_F, CHUNK) = gelu(W1^T X + b1)
                    ht = [hpool.tile([P, CHUNK], BF16, tag=f"h{f}", name=f"ht{f}")
                          for f in range(FP)]
                    for f in range(FP):
                        ph = psum_h.tile([P, CHUNK], FP32, tag="ph")
                        for d in range(DP):
                            nc.tensor.matmul(
                                ph[:], w1t[d][:, bass.ts(f, P)], xt[d][:],
                                start=(d == 0), stop=(d == DP - 1),
                            )
                        nc.scalar.activation(ht[f][:], ph[:], gelu, bias=b1t[:, f:f + 1])
                    # Y^T tiles: y[dout] (128_D, CHUNK) = W2^T H + b2
                    for dout in range(DP):
                        py = psum_y.tile([P, CHUNK], FP32, tag="py")
                        for f in range(FP):
                            nc.tensor.matmul(
                                py[:], w2t[f][:, bass.ts(dout, P)], ht[f][:],
                                start=(f == 0), stop=(f == FP - 1),
                            )
                        yt_sb = ypool.tile([P, CHUNK], BF16, tag="y")
                        nc.vector.tensor_scalar_add(yt_sb[:], py[:], b2t[:, dout:dout + 1])
                        nc.sync.dma_start(yT[dout, :, bass.ts(c, CHUNK)], yt_sb[:])
    return nc


# ---------------------------------------------------------------- host glue
_cache = {}
last_exec_ns = {}


def _get_programs():
    if "router" not in _cache:
        _cache["router"] = build_router()
        _cache["router"].finalize()
        _cache["experts"] = build_experts()
        _cache["experts"].finalize()
    return _cache["router"], _cache["experts"]


def _run(nc, in_maps, trace, label):
    kw = {}
    if trace:
        import tempfile
        kw = dict(trace=True, tmpdir=tempfile.mkdtemp())
    res = run_bass_kernel_spmd(nc, in_maps, list(range(N_CORES)), **kw)
    if trace:
        last_exec_ns[label] = res.exec_time_ns
    return res.results


def _gelu_tanh(z):
    return 0.5 * z * (1.0 + np.tanh(np.sqrt(2.0 / np.pi) * (z + 0.044715 * z ** 3)))


def moe_forward(x, Wr, W1, b1, W2, b2, trace=False):
    nc_router, nc_experts = _get_programs()

    x = np.ascontiguousarray(x, dtype=np.float32)
    xf = x.reshape(NT, D)

    # ---- launch 1: router logits (fp32 so top-k selection matches fp32 ref)
    wr_in = np.ascontiguousarray(np.asarray(Wr, dtype=np.float32).reshape(DP, P, E))
    in_maps = []
    for m in range(N_CORES):
        shard = xf[m * TOK_PER_CORE:(m + 1) * TOK_PER_CORE]
        in_maps.append({
            "xT": np.ascontiguousarray(shard.T.reshape(DP, P, TOK_PER_CORE)),
            "wr": wr_in,
        })
    res = _run(nc_router, in_maps, trace, "router")
    logits = np.concatenate([r["logitsT"].T for r in res], axis=0)  # (NT, E) f32

    # ---- host: softmax / top-2 / gates / aux loss ----
    lmax = logits.max(axis=1, keepdims=True)
    ex = np.exp(logits - lmax)
    probs = ex / ex.sum(axis=1, keepdims=True)
    a = np.argmax(logits, axis=1)            # top-1 (ties -> lowest index,
    l2 = logits.copy()                       #  matching jax.lax.top_k)
    l2[np.arange(NT), a] = -np.inf
    b_ = np.argmax(l2, axis=1)               # top-2
    top_i = np.stack([a, b_], axis=1)
    tv = np.take_along_axis(logits, top_i, axis=1)
    ge = np.exp(tv - tv.max(axis=1, keepdims=True))
    gates = (ge / ge.sum(axis=1, keepdims=True)).astype(np.float32)

    f_frac = np.bincount(a, minlength=E).astype(np.float32) / np.float32(NT)
    P_mean = probs.mean(axis=0, dtype=np.float64).astype(np.float32)
    aux_loss = np.float32(E * np.float32(np.sum(f_frac * P_mean)))

    # ---- host: dispatch (gather tokens per expert) ----
    order = np.argsort(top_i.ravel(), kind="stable")
    tok_of = order // K
    counts = np.bincount(top_i.ravel(), minlength=E)
    starts = np.concatenate([[0], np.cumsum(counts)])
    idx_lists = [tok_of[starts[e]:starts[e + 1]] for e in range(E)]

    xf_bf = xf.astype(ml_dtypes.bfloat16)
    # expert -> (core, slot): largest 8 in slot 0, smallest 8 in slot 1
    by_size = np.argsort(-counts, kind="stable")
    assign = {}
    for m in range(N_CORES):
        assign[by_size[m]] = (m, 0)
        assign[by_size[2 * N_CORES - 1 - m]] = (m, 1)

    w1_bf = np.asarray(W1).astype(ml_dtypes.bfloat16)
    w2_bf = np.asarray(W2).astype(ml_dtypes.bfloat16)
    b1f = np.ascontiguousarray(b1, dtype=np.float32)
    b2f = np.ascontiguousarray(b2, dtype=np.float32)

    in_maps = []
    for m in range(N_CORES):
        im = {"w1": np.empty((EXP_PER_CORE, DP, P, F), dtype=ml_dtypes.bfloat16),
              "w2": np.empty((EXP_PER_CORE, FP, P, D), dtype=ml_dtypes.bfloat16),
              "b1r": np.empty((EXP_PER_CORE, P, FP), dtype=np.float32),
              "b2r": np.empty((EXP_PER_CORE, P, DP), dtype=np.float32)}
        for sl, cap in enumerate(CAPS):
            im[f"xg{sl}"] = np.zeros((D, cap), dtype=ml_dtypes.bfloat16)
        in_maps.append(im)
    overflow = []
    for e in range(E):
        m, s = assign[e]
        idx = idx_lists[e]
        if len(idx) > CAPS[s]:
            overflow.append((e, idx[CAPS[s]:]))
            idx = idx[:CAPS[s]]
            idx_lists[e] = idx
        im = in_maps[m]
        im[f"xg{s}"][:, :len(idx)] = xf_bf[idx].T
        im["w1"][s] = w1_bf[e].reshape(DP, P, F)
        im["w2"][s] = w2_bf[e].reshape(FP, P, D)
        im["b1r"][s] = b1f[e].reshape(FP, P).T
        im["b2r"][s] = b2f[e].reshape(DP, P).T
    for im in in_maps:
        for sl, cap in enumerate(CAPS):
            im[f"xg{sl}"] = np.ascontiguousarray(im[f"xg{sl}"].reshape(DP, P, cap))

    res = _run(nc_experts, in_maps, trace, "experts")

    # ---- host: combine (scatter-add with gates) ----
    gate_of = np.zeros((NT, E), dtype=np.float32)
    gate_of[np.arange(NT), top_i[:, 0]] = gates[:, 0]
    gate_of[np.arange(NT), top_i[:, 1]] = gates[:, 1]
    out = np.zeros((NT, D), dtype=np.float32)
    for e in range(E):
        m, s = assign[e]
        idx = idx_lists[e]
        y = np.asarray(res[m][f"yT{s}"].reshape(D, CAPS[s])[:, :len(idx)],
                       dtype=np.float32)
        out[idx] += gate_of[idx, e][:, None] * y.T
    # capacity-overflow fallback: exact host compute for the excess tokens
    for e, idx in overflow:
        z = xf[idx] @ np.asarray(W1[e], dtype=np.float32) + np.asarray(b1, np.float32)[e]
        y = _gelu_tanh(z) @ np.asarray(W2[e], dtype=np.float32) + np.asarray(b2, np.float32)[e]
        out[idx] += gate_of[idx, e][:, None] * y

    return out.reshape(B, T, D), aux_loss


def kernel(x, Wr, W1, b1, W2, b2):
    trace = os.environ.get("MOE_KERNEL_TRACE", "0") == "1"
    return moe_forward(x, Wr, W1, b1, W2, b2, trace=trace)
